# revision 25
# baseline (speedup 1.0000x reference)
"""Trainium2 Bass kernel for nn_Aug_MLP_P_RPY_sincos_unc_indep.

Data-parallel over 8 NeuronCores (batch 16384 -> 2048/core).
Per core, everything runs in a transposed layout (features on SBUF
partitions, batch on the free dim, 4 tiles of 512 samples):

  forward:   Z1^T = W0^T q^T   (lhsT = W0 natural)   H = sigmoid(Z + b)
             Z2^T = W1^T H1^T  (lhsT = W1 natural)
             P    = W2^T H2^T  (lhsT = W2 natural)
  jacobian (from the small output side, o = output index):
             V_o  = D2 * W2[:,o] (* head scale)      D = (h-1)*h  [negated
             Y_o  = W1 V_o        (lhsT = W1^T)       twice -> cancels]
             Z_o  = Y_o * D1
             J_o  = W0 Z_o        (lhsT = W0^T)  -> (7, batch) rows of J
  rpy heads: sin/cos via range-wrap + polynomials in u = x^2 (DVE/GPSIMD,
             keeps ACT on one table set); atan2 via Arctan + Sign;
             all row gather/scatter/broadcast via small selector matmuls
             (PE), since compute engines need 32-aligned partition bases.
  vel:       J @ qdot elementwise (quadrant-packed) + selector reduce.

Matmuls run in float32r (fp32 storage, ~13-bit mantissa products,
fp32 PSUM accumulation) at ~4x the speed of plain fp32.
"""

import numpy as np

NCORES = 8
M = 16384
MC = M // NCORES          # per-core batch
S = 512                   # batch tile (free dim)
NT = MC // S
PI = float(np.pi)
HEADS = ("roll", "pitch", "yaw")

_COEFS = None


def _trig_coefs():
    """sin(x) = x*P(u), cos(x) = Q(u), u = x^2, on [-pi, pi]; ~1e-9 abs."""
    global _COEFS
    if _COEFS is None:
        x = (np.pi / 2) * (1 - np.cos(np.linspace(0, np.pi, 4001)))[1:]
        u = x * x
        A = np.stack([u ** k for k in range(9)], axis=1)
        ps, *_ = np.linalg.lstsq(A * x[:, None], np.sin(x) / x * x, rcond=None)
        qc, *_ = np.linalg.lstsq(A, np.cos(x), rcond=None)
        _COEFS = ([float(c) for c in ps], [float(c) for c in qc])
    return _COEFS


def _build(reps=1):
    import concourse.tile as tile
    import concourse.mybir as mybir
    from concourse import bacc

    F32 = mybir.dt.float32
    F32R = mybir.dt.float32r
    AF = mybir.ActivationFunctionType
    ALU = mybir.AluOpType
    import concourse.bass as bass

    nc = bacc.Bacc("TRN2", target_bir_lowering=False, debug=False,
                   num_devices=NCORES)

    dram = {}

    def din(name, shape):
        dram[name] = nc.dram_tensor(name, list(shape), F32,
                                    kind="ExternalInput").ap()

    def dout(name, shape):
        dram[name] = nc.dram_tensor(name, list(shape), F32,
                                    kind="ExternalOutput").ap()

    din("qT", (7, MC))
    din("qd4", (128, MC))
    din("pw0", (7, 512)), din("pw0t", (128, 4, 7)), din("pw1", (128, 4, 512))
    din("pw1t", (128, 4, 512)), din("pw2", (128, 4, 3))
    din("pb0", (128, 4)), din("pb1", (128, 4)), din("pb2", (3, 1))
    for h in HEADS:
        din(h + "w0", (7, 256)), din(h + "w0t", (128, 2, 7))
        din(h + "w1", (128, 2, 256)), din(h + "w1t", (128, 2, 256))
        din(h + "w2", (128, 2, 2))
        din(h + "b0", (128, 2)), din(h + "b1", (128, 2)), din(h + "b2", (2, 1))
    din("sely", (2, 18))       # per-head (2,6): y rows -> ypack rows
    din("selbc", (6, 384))     # per-head (6,128): bcast cos6[hi]
    din("selbn", (6, 384))     # per-head (6,128): bcast -sin6[3+hi]
    din("sel7", (3, 213))      # per-head (3,71): bcast row hi -> quadrant rows
    din("blkA", (71, 9)), din("blkB", (71, 9)), din("blkC", (71, 9))
    dout("outT", (18, MC))
    dout("jtT", (63, MC))
    dout("outaT", (6, MC))
    dout("jaT", (42, MC))

    PC, QC = _trig_coefs()

    def _sl(shape):
        return tuple(slice(None) for _ in shape)

    with tile.TileContext(nc) as tc:
        with (
            tc.tile_pool(name="wp", bufs=1) as wp,
            tc.tile_pool(name="io", bufs=1) as io,
        ):
            # -------- weights: load fp32, convert once to fp32r --------
            W = {}
            with tc.tile_pool(name="stg", bufs=1) as stg:
                def load_r(name, shape, pool=wp):
                    st = stg.tile(list(shape), F32, tag="stg")
                    nc.sync.dma_start(st[_sl(shape)], dram[name][_sl(shape)])
                    w = pool.tile(list(shape), F32R, tag=name)
                    nc.vector.tensor_copy(w[_sl(shape)], st[_sl(shape)])
                    W[name] = w

                def load_f(name, shape):
                    w = wp.tile(list(shape), F32, tag=name)
                    nc.sync.dma_start(w[_sl(shape)], dram[name][_sl(shape)])
                    W[name] = w

                load_r("pw0", (7, 512))
                load_r("qT", (7, MC), pool=io)
                load_r("pw1", (128, 4, 512))
                for h in HEADS:
                    load_r(h + "w0", (7, 256))
                    load_r(h + "w1", (128, 2, 256))
                load_r("pw1t", (128, 4, 512))
                load_r("pw0t", (128, 4, 7))
                load_r("pw2", (128, 4, 3))
                for h in HEADS:
                    load_r(h + "w1t", (128, 2, 256))
                    load_r(h + "w0t", (128, 2, 7))
                    load_r(h + "w2", (128, 2, 2))
                for n in ("sely", "selbc", "selbn", "sel7",
                          "blkA", "blkB", "blkC"):
                    load_r(n, dram[n].shape)
                load_f("pb0", (128, 4)), load_f("pb1", (128, 4))
                load_f("pb2", (3, 1))
                for h in HEADS:
                    load_f(h + "b0", (128, 2)), load_f(h + "b1", (128, 2))
                    load_f(h + "b2", (2, 1))
                load_f("qd4", (128, MC))

            # persistent J staging (quadrants 0/32/64 hold row blocks)
            jt = [io.tile([128, S], F32, tag=f"jt{i}", name=f"jt{i}")
                  for i in range(3)]
            jq = [io.tile([128, S], F32R, tag=f"jq{i}", name=f"jq{i}")
                  for i in range(3)]
            jaQ = io.tile([128, S], F32, tag="jaQ")
            for i in range(3):
                nc.vector.memset(jt[i][:, :], 0.0)

            NETS = {"p": dict(pre="p", C=4, O=3)}
            for h in HEADS:
                NETS[h] = dict(pre=h, C=2, O=2)

            with (
                tc.tile_pool(name="hd", bufs=1) as hd,
                tc.tile_pool(name="hd1", bufs=1) as hd1,
                tc.tile_pool(name="vz", bufs=2) as vz,
                tc.tile_pool(name="vc", bufs=2) as vc,
                tc.tile_pool(name="sm", bufs=1) as sm,
                tc.tile_pool(name="sm1", bufs=1) as sm1,
                tc.tile_pool(name="psA", bufs=4, space="PSUM") as psA,
                tc.tile_pool(name="psR", bufs=4, space="PSUM") as psR,
            ):
                def tile_body(t):
                    ts = slice(t * S, (t + 1) * S)
                    qr = W["qT"][:, ts]

                    def fwd(p):
                        net = NETS[p]
                        C, O = net["C"], net["O"]
                        pre = net["pre"]
                        w0, w1, w2 = W[pre + "w0"], W[pre + "w1"], W[pre + "w2"]
                        b0, b1 = W[pre + "b0"], W[pre + "b1"]
                        h1tag = "rh1" if pre != "p" else "ph1"
                        H1 = hd.tile([128, C, S], F32R, tag=h1tag)
                        D1 = hd1.tile([128, C, S], F32, tag=pre + "d1")
                        for jc in range(C):
                            z = psA.tile([128, S], F32, tag="psA")
                            nc.tensor.matmul(z[:, :],
                                             w0[0:7, jc * 128:(jc + 1) * 128],
                                             qr, start=True, stop=True)
                            nc.scalar.activation(H1[:, jc, :], z[:, :],
                                                 AF.Sigmoid,
                                                 bias=b0[:, jc:jc + 1])
                            nc.vector.scalar_tensor_tensor(
                                D1[:, jc, :], H1[:, jc, :], 1.0, H1[:, jc, :],
                                op0=ALU.subtract, op1=ALU.mult)
                        D2 = hd1.tile([128, C, S], F32, tag=pre + "d2")
                        ph = psR.tile([O, S], F32, tag="psR")
                        for kc in range(C):
                            z = psA.tile([128, S], F32, tag="psA")
                            for jc in range(C):
                                nc.tensor.matmul(
                                    z[:, :], w1[:, jc, kc * 128:(kc + 1) * 128],
                                    H1[:, jc, :], start=(jc == 0),
                                    stop=(jc == C - 1))
                            h2 = vz.tile([128, S], F32R, tag=pre + "h2")
                            nc.scalar.activation(h2[:, :], z[:, :], AF.Sigmoid,
                                                 bias=b1[:, kc:kc + 1])
                            nc.tensor.matmul(ph[:, :], w2[:, kc, 0:O],
                                             h2[:, :], start=(kc == 0),
                                             stop=(kc == C - 1))
                            nc.vector.scalar_tensor_tensor(
                                D2[:, kc, :], h2[:, :], 1.0, h2[:, :],
                                op0=ALU.subtract, op1=ALU.mult)
                        return D1, D2, ph

                    def bwd_stream(p, o, D1, D2, scale_ps):
                        net = NETS[p]
                        C, pre = net["C"], net["pre"]
                        w1t, w0t = W[pre + "w1t"], W[pre + "w0t"]
                        w2 = W[pre + "w2"]
                        vtag = "rvc" if pre != "p" else "pvc"
                        ztag = "rz" if pre != "p" else "pz"
                        Vs = []
                        for kc in range(C):
                            wcol = w2[:, kc, o:o + 1].bitcast(F32)
                            V = vc.tile([128, S], F32R, tag=vtag,
                                        bufs=(5 if pre == "p" else 3))
                            if scale_ps is None:
                                nc.gpsimd.tensor_scalar_mul(
                                    V[:, :], D2[:, kc, :], wcol)
                            else:
                                nc.vector.scalar_tensor_tensor(
                                    V[:, :], D2[:, kc, :], wcol,
                                    scale_ps[:, :], op0=ALU.mult, op1=ALU.mult)
                            Vs.append(V)
                        jr = psR.tile([7, S], F32, tag="psR")
                        for jc in range(C):
                            y = psA.tile([128, S], F32, tag="psA")
                            for kc in range(C):
                                nc.tensor.matmul(
                                    y[:, :],
                                    w1t[:, kc, jc * 128:(jc + 1) * 128],
                                    Vs[kc][:, :], start=(kc == 0),
                                    stop=(kc == C - 1))
                            zc = vc.tile([128, S], F32R, tag=ztag)
                            nc.vector.tensor_tensor(zc[:, :], y[:, :],
                                                    D1[:, jc, :], op=ALU.mult)
                            nc.tensor.matmul(jr[:, :], w0t[:, jc, 0:7],
                                             zc[:, :], start=(jc == 0),
                                             stop=(jc == C - 1))
                        return jr

                    # ---------------- pos net ----------------
                    D1, D2, ph = fwd("p")
                    p3 = sm1.tile([3, S], F32, tag="p3")
                    nc.scalar.activation(p3[:, :], ph[:, :], AF.Identity,
                                         bias=W["pb2"][:, 0:1])
                    nc.sync.dma_start(dram["outT"][0:3, ts], p3[:, :])
                    nc.sync.dma_start(dram["outaT"][0:3, ts], p3[:, :])
                    for o in range(3):
                        jr = bwd_stream("p", o, D1, D2, None)
                        nc.scalar.copy(jt[0][32 * o:32 * o + 7, :], jr[:, :])

                    # ------------- rpy forward + batched trig -------------
                    rpy_fwd = {}
                    ypack = psR.tile([6, S], F32, tag="psR")
                    for hi, h in enumerate(HEADS):
                        D1h, D2h, phh = fwd(h)
                        rpy_fwd[h] = (D1h, D2h)
                        yh = sm1.tile([2, S], F32R, tag="yh")
                        nc.scalar.activation(yh[:, :], phh[:, :], AF.Identity,
                                             bias=W[h + "b2"][:, 0:1])
                        nc.tensor.matmul(ypack[:, :],
                                         W["sely"][:, 6 * hi:6 * hi + 6],
                                         yh[:, :], start=(hi == 0),
                                         stop=(hi == 2))
                    y6 = sm1.tile([6, S], F32, tag="y6")
                    nc.scalar.copy(y6[:, :], ypack[:, :])
                    yws = sm1.tile([6, S], F32, tag="yws")
                    nc.vector.add_range_wrap(yws[:, :], y6[:, :], 0.0, PI,
                                             2 * PI)
                    u6 = sm1.tile([6, S], F32, tag="u6")
                    nc.scalar.activation(u6[:, :], yws[:, :], AF.Square)
                    qq = [sm1.tile([6, S], F32, tag="qq0", name="qq0"),
                          sm1.tile([6, S], F32, tag="qq1", name="qq1")]
                    nc.vector.tensor_scalar_mul(qq[0][:, :], u6[:, :], QC[8])
                    b = 0
                    for k in range(7, 0, -1):
                        nc.vector.scalar_tensor_tensor(
                            qq[1 - b][:, :], qq[b][:, :], QC[k], u6[:, :],
                            op0=ALU.add, op1=ALU.mult)
                        b = 1 - b
                    # cos6: rows 0-2 cos(y0), rows 3-5 cos(y1)=c
                    cos6 = sm.tile([6, S], F32R, tag="cos6")
                    nc.vector.tensor_scalar(cos6[:, :], qq[b][:, :], QC[0],
                                            None, op0=ALU.add)
                    pp = [sm1.tile([6, S], F32, tag="qq0", name="pp0"),
                          sm1.tile([6, S], F32, tag="qq1", name="pp1")]
                    nc.vector.tensor_scalar_mul(pp[0][:, :], u6[:, :], PC[8])
                    a = 0
                    for k in range(7, 0, -1):
                        nc.vector.scalar_tensor_tensor(
                            pp[1 - a][:, :], pp[a][:, :], PC[k], u6[:, :],
                            op0=ALU.add, op1=ALU.mult)
                        a = 1 - a
                    # sin6: rows 0-2 sin(y0)=s, rows 3-5 sin(y1)
                    sin6 = sm.tile([6, S], F32R, tag="sin6")
                    nc.vector.scalar_tensor_tensor(sin6[:, :], pp[a][:, :],
                                                   PC[0], yws[:, :],
                                                   op0=ALU.add, op1=ALU.mult)
                    s3f = sin6[0:3, :].bitcast(F32)
                    nc.sync.dma_start(dram["outT"][3:6, ts],
                                      sin6[0:3, :].bitcast(F32))
                    nc.sync.dma_start(dram["outT"][6:9, ts],
                                      cos6[3:6, :].bitcast(F32))
                    c3f_t = sm1.tile([3, S], F32, tag="c3f")
                    nc.sync.dma_start(c3f_t[:, :],
                                      cos6[3:6, :].bitcast(F32))
                    c3f = c3f_t[:, :]

                    # angles: atan2(s, c) = atan(s/c) + pi*sign(s)*[c<0]
                    ssq = sm1.tile([3, S], F32, tag="t3a", name="ssq")
                    csq = sm1.tile([3, S], F32, tag="t3b", name="csq")
                    nc.scalar.activation(ssq[:, :], s3f, AF.Square)
                    nc.scalar.activation(csq[:, :], c3f, AF.Square)
                    rden = sm1.tile([3, S], F32, tag="t3c", name="rden")
                    nc.vector.tensor_tensor(rden[:, :], ssq[:, :], csq[:, :],
                                            op=ALU.add)
                    rinv = sm1.tile([3, S], F32, tag="rinv")
                    nc.vector.reciprocal(rinv[:, :], rden[:, :])
                    crec = sm1.tile([3, S], F32, tag="t3a", name="crec")
                    nc.vector.reciprocal(crec[:, :], c3f)
                    q3v = sm1.tile([3, S], F32, tag="t3b", name="q3v")
                    nc.vector.tensor_tensor(q3v[:, :], s3f, crec[:, :],
                                            op=ALU.mult)
                    at3 = sm1.tile([3, S], F32, tag="t3a", name="at3")
                    nc.scalar.activation(at3[:, :], q3v[:, :], AF.Arctan)
                    sgn3 = sm1.tile([3, S], F32, tag="t3c", name="sgn3")
                    nc.scalar.activation(sgn3[:, :], s3f, AF.Sign)
                    cneg = sm1.tile([3, S], F32, tag="t3d", name="cneg")
                    nc.vector.tensor_scalar(cneg[:, :], c3f, 0.0, None,
                                            op0=ALU.is_lt)
                    adj3 = sm1.tile([3, S], F32, tag="t3b", name="adj3")
                    nc.vector.scalar_tensor_tensor(adj3[:, :], sgn3[:, :], PI,
                                                   cneg[:, :], op0=ALU.mult,
                                                   op1=ALU.mult)
                    ang = sm1.tile([3, S], F32, tag="t3c", name="ang")
                    nc.vector.tensor_tensor(ang[:, :], at3[:, :], adj3[:, :],
                                            op=ALU.add)
                    nc.sync.dma_start(dram["outaT"][3:6, ts], ang[:, :])
                    cr3 = sm1.tile([3, S], F32R, tag="cr3")
                    sr3 = sm1.tile([3, S], F32R, tag="sr3")
                    nc.vector.tensor_tensor(cr3[:, :], c3f, rinv[:, :],
                                            op=ALU.mult)
                    nc.vector.tensor_tensor(sr3[:, :], s3f, rinv[:, :],
                                            op=ALU.mult)

                    # ---------------- rpy backward ----------------
                    tmpA = sm1.tile([128, S], F32, tag="tmpA")
                    tmpB = sm1.tile([128, S], F32, tag="tmpB")
                    for hi, h in enumerate(HEADS):
                        D1h, D2h = rpy_fwd[h]
                        qb = 32 * hi
                        c0b = psA.tile([128, S], F32, tag="psA")
                        nc.tensor.matmul(c0b[:, :],
                                         W["selbc"][:, 128 * hi:128 * hi + 128],
                                         cos6[:, :], start=True, stop=True)
                        s1b = psA.tile([128, S], F32, tag="psA")
                        nc.tensor.matmul(s1b[:, :],
                                         W["selbn"][:, 128 * hi:128 * hi + 128],
                                         sin6[:, :], start=True, stop=True)
                        jr_sin = bwd_stream(h, 0, D1h, D2h, c0b)
                        nc.scalar.copy(jt[1][qb:qb + 7, :], jr_sin[:, :])
                        crb = psR.tile([128, S], F32, tag="psR")
                        nc.tensor.matmul(crb[0:71, :],
                                         W["sel7"][:, 71 * hi:71 * hi + 71],
                                         cr3[:, :], start=True, stop=True)
                        nc.vector.tensor_tensor(tmpA[qb:qb + 7, :],
                                                jt[1][qb:qb + 7, :],
                                                crb[qb:qb + 7, :], op=ALU.mult)
                        jr_cos = bwd_stream(h, 1, D1h, D2h, s1b)
                        nc.scalar.copy(jt[2][qb:qb + 7, :], jr_cos[:, :])
                        srb = psR.tile([128, S], F32, tag="psR")
                        nc.tensor.matmul(srb[0:71, :],
                                         W["sel7"][:, 71 * hi:71 * hi + 71],
                                         sr3[:, :], start=True, stop=True)
                        nc.vector.tensor_tensor(tmpB[qb:qb + 7, :],
                                                jt[2][qb:qb + 7, :],
                                                srb[qb:qb + 7, :], op=ALU.mult)
                        nc.vector.tensor_tensor(jaQ[qb:qb + 7, :],
                                                tmpA[qb:qb + 7, :],
                                                tmpB[qb:qb + 7, :],
                                                op=ALU.subtract)

                    # ---------------- vel + output DMAs ----------------
                    for i in range(3):
                        nc.gpsimd.tensor_tensor(jq[i][0:71, :],
                                                jt[i][0:71, :],
                                                W["qd4"][0:71, ts],
                                                op=ALU.mult)
                    vel = psR.tile([9, S], F32, tag="psR")
                    for i, bname in enumerate(("blkA", "blkB", "blkC")):
                        nc.tensor.matmul(vel[:, :], W[bname][:, 0:9],
                                         jq[i][0:71, :], start=(i == 0),
                                         stop=(i == 2))
                    velf = sm1.tile([9, S], F32, tag="p3", name="velf")
                    nc.scalar.copy(velf[:, :], vel[:, :])
                    nc.sync.dma_start(dram["outT"][9:18, ts], velf[:, :])
                    for q in range(3):
                        sl = jt[0][32 * q:32 * q + 7, :]
                        nc.sync.dma_start(
                            dram["jtT"][7 * q:7 * q + 7, ts], sl)
                        nc.sync.dma_start(
                            dram["jaT"][7 * q:7 * q + 7, ts], sl)
                        nc.sync.dma_start(
                            dram["jtT"][21 + 7 * q:28 + 7 * q, ts],
                            jt[1][32 * q:32 * q + 7, :])
                        nc.sync.dma_start(
                            dram["jtT"][42 + 7 * q:49 + 7 * q, ts],
                            jt[2][32 * q:32 * q + 7, :])
                        nc.sync.dma_start(
                            dram["jaT"][21 + 7 * q:28 + 7 * q, ts],
                            jaQ[32 * q:32 * q + 7, :])

                if reps > 1:
                    with tc.For_i(0, reps, 1):
                        for t in range(NT):
                            tile_body(t)
                else:
                    for t in range(NT):
                        tile_body(t)

    nc.compile()
    return nc


def _host_inputs(inputs):
    """Prepare per-core in_maps: shard x, replicate + re-lay-out weights."""
    def chunked(a, c):
        f = a.shape[1]
        return np.ascontiguousarray(
            a.reshape(c, 128, f).transpose(1, 0, 2)).astype(np.float32)

    com = {}
    for p, pre, d, o in (("pos", "p", 512, 3),) + tuple(
            (h, h, 256, 2) for h in HEADS):
        W0 = np.asarray(inputs[p + "_W0"], np.float32)
        W1 = np.asarray(inputs[p + "_W1"], np.float32)
        W2 = np.asarray(inputs[p + "_W2"], np.float32)
        b0 = np.asarray(inputs[p + "_b0"], np.float32)
        b1 = np.asarray(inputs[p + "_b1"], np.float32)
        b2 = np.asarray(inputs[p + "_b2"], np.float32)
        c = d // 128
        com[pre + "w0"] = np.ascontiguousarray(W0)
        com[pre + "w0t"] = chunked(np.ascontiguousarray(W0.T), c)
        com[pre + "w1"] = chunked(W1, c)
        com[pre + "w1t"] = chunked(np.ascontiguousarray(W1.T), c)
        com[pre + "w2"] = chunked(W2, c)
        com[pre + "b0"] = np.ascontiguousarray(b0.reshape(c, 128).T)
        com[pre + "b1"] = np.ascontiguousarray(b1.reshape(c, 128).T)
        com[pre + "b2"] = b2.reshape(o, 1).copy()

    sely = np.zeros((2, 18), np.float32)
    selbc = np.zeros((6, 384), np.float32)
    selbn = np.zeros((6, 384), np.float32)
    sel7 = np.zeros((3, 213), np.float32)
    for hi in range(3):
        sely[0, 6 * hi + hi] = 1.0
        sely[1, 6 * hi + 3 + hi] = 1.0
        selbc[hi, 128 * hi:128 * hi + 128] = 1.0
        selbn[3 + hi, 128 * hi:128 * hi + 128] = -1.0
        sel7[hi, 71 * hi + 32 * hi:71 * hi + 32 * hi + 7] = 1.0
    com["sely"], com["selbc"], com["selbn"], com["sel7"] = (sely, selbc,
                                                            selbn, sel7)
    for nm, gs in (("blkA", (0, 1, 2)), ("blkB", (3, 4, 5)),
                   ("blkC", (6, 7, 8))):
        blk = np.zeros((71, 9), np.float32)
        for q, g in enumerate(gs):
            blk[32 * q:32 * q + 7, g] = 1.0
        com[nm] = blk

    x = np.asarray(inputs["x"], np.float32)
    in_maps = []
    for cix in range(NCORES):
        xs = x[cix * MC:(cix + 1) * MC]            # (MC, 14)
        m = dict(com)
        m["qT"] = np.ascontiguousarray(xs[:, 0:7].T)
        qd4 = np.zeros((128, MC), np.float32)
        for q in range(4):
            qd4[32 * q:32 * q + 7] = xs[:, 7:14].T
        m["qd4"] = qd4
        in_maps.append(m)
    return in_maps


_PROGRAM = None


def _get_program():
    global _PROGRAM
    if _PROGRAM is None:
        _PROGRAM = _build(reps=1)
    return _PROGRAM


def run_spmd(nc, in_maps):
    from concourse.bass_utils import run_bass_kernel_spmd
    return run_bass_kernel_spmd(nc, in_maps, core_ids=list(range(NCORES)))


def _gather(results):
    out = np.concatenate([r["outT"].T for r in results], axis=0)
    j_tot = np.concatenate([r["jtT"].T for r in results], axis=0)
    j_tot = j_tot.reshape(M, 9, 7)
    out_ang = np.concatenate([r["outaT"].T for r in results], axis=0)
    j_ang = np.concatenate([r["jaT"].T for r in results], axis=0)
    j_ang = j_ang.reshape(M, 6, 7)
    return out, j_tot, out_ang, j_ang


def kernel(**inputs):
    nc = _get_program()
    in_maps = _host_inputs(inputs)
    res = run_spmd(nc, in_maps)
    return _gather(res.results)


# revision 27
# speedup vs baseline: 36.7615x; 36.7615x over previous
"""Trainium2 Bass kernel for nn_Aug_MLP_P_RPY_sincos_unc_indep.

Data-parallel over 8 NeuronCores (batch 16384 -> 2048/core).
Per core, everything runs in a transposed layout (features on SBUF
partitions, batch on the free dim, 4 tiles of 512 samples):

  forward:   Z1^T = W0^T q^T   (lhsT = W0 natural)   H = sigmoid(Z + b)
             Z2^T = W1^T H1^T  (lhsT = W1 natural)
             P    = W2^T H2^T  (lhsT = W2 natural)
  jacobian (from the small output side, o = output index):
             V_o  = D2 * W2[:,o] (* head scale)      D = (h-1)*h  [negated
             Y_o  = W1 V_o        (lhsT = W1^T)       twice -> cancels]
             Z_o  = Y_o * D1
             J_o  = W0 Z_o        (lhsT = W0^T)  -> (7, batch) rows of J
  rpy heads: sin/cos via range-wrap + polynomials in u = x^2 (DVE/GPSIMD,
             keeps ACT on one table set); atan2 via Arctan + Sign;
             all row gather/scatter/broadcast via small selector matmuls
             (PE), since compute engines need 32-aligned partition bases.
  vel:       J @ qdot elementwise (quadrant-packed) + selector reduce.

Matmuls run in float32r (fp32 storage, ~13-bit mantissa products,
fp32 PSUM accumulation) at ~4x the speed of plain fp32.
"""

import numpy as np

NCORES = 8
M = 16384
MC = M // NCORES          # per-core batch
S = 512                   # batch tile (free dim)
NT = MC // S
PI = float(np.pi)
HEADS = ("roll", "pitch", "yaw")

_COEFS = None


def _trig_coefs():
    """sin(x) = x*P(u), cos(x) = Q(u), u = x^2, on [-pi, pi]; ~1e-9 abs."""
    global _COEFS
    if _COEFS is None:
        x = (np.pi / 2) * (1 - np.cos(np.linspace(0, np.pi, 4001)))[1:]
        u = x * x
        A = np.stack([u ** k for k in range(9)], axis=1)
        ps, *_ = np.linalg.lstsq(A * x[:, None], np.sin(x) / x * x, rcond=None)
        qc, *_ = np.linalg.lstsq(A, np.cos(x), rcond=None)
        _COEFS = ([float(c) for c in ps], [float(c) for c in qc])
    return _COEFS


def _build(reps=1):
    import concourse.tile as tile
    import concourse.mybir as mybir
    from concourse import bacc

    F32 = mybir.dt.float32
    F32R = mybir.dt.float32r
    AF = mybir.ActivationFunctionType
    ALU = mybir.AluOpType
    import concourse.bass as bass

    nc = bacc.Bacc("TRN2", target_bir_lowering=False, debug=False,
                   num_devices=NCORES)

    dram = {}

    def din(name, shape):
        dram[name] = nc.dram_tensor(name, list(shape), F32,
                                    kind="ExternalInput").ap()

    def dout(name, shape):
        dram[name] = nc.dram_tensor(name, list(shape), F32,
                                    kind="ExternalOutput").ap()

    din("qT", (7, MC))
    din("qd4", (128, MC))
    din("pw0", (7, 512)), din("pw0t", (128, 4, 7)), din("pw1", (128, 4, 512))
    din("pw1t", (128, 4, 512)), din("pw2", (128, 4, 3))
    din("pb0", (128, 4)), din("pb1", (128, 4)), din("pb2", (3, 1))
    for h in HEADS:
        din(h + "w0", (7, 256)), din(h + "w0t", (128, 2, 7))
        din(h + "w1", (128, 2, 256)), din(h + "w1t", (128, 2, 256))
        din(h + "w2", (128, 2, 2))
        din(h + "b0", (128, 2)), din(h + "b1", (128, 2)), din(h + "b2", (2, 1))
    din("sely", (2, 18))       # per-head (2,6): y rows -> ypack rows
    din("selbc", (6, 384))     # per-head (6,128): bcast cos6[hi]
    din("selbn", (6, 384))     # per-head (6,128): bcast -sin6[3+hi]
    din("sel7", (3, 213))      # per-head (3,71): bcast row hi -> quadrant rows
    din("blkA", (71, 9)), din("blkB", (71, 9)), din("blkC", (71, 9))
    dout("outT", (18, MC))
    dout("jtT", (63, MC))
    dout("outaT", (6, MC))
    dout("jaT", (42, MC))

    PC, QC = _trig_coefs()

    def _sl(shape):
        return tuple(slice(None) for _ in shape)

    with tile.TileContext(nc) as tc:
        with (
            tc.tile_pool(name="wp", bufs=1) as wp,
            tc.tile_pool(name="io", bufs=1) as io,
        ):
            # -------- weights: load fp32, convert once to fp32r --------
            W = {}
            with tc.tile_pool(name="stg", bufs=1) as stg:
                def load_r(name, shape, pool=wp):
                    st = stg.tile(list(shape), F32, tag="stg")
                    nc.sync.dma_start(st[_sl(shape)], dram[name][_sl(shape)])
                    w = pool.tile(list(shape), F32R, tag=name)
                    nc.vector.tensor_copy(w[_sl(shape)], st[_sl(shape)])
                    W[name] = w

                def load_f(name, shape):
                    w = wp.tile(list(shape), F32, tag=name)
                    nc.sync.dma_start(w[_sl(shape)], dram[name][_sl(shape)])
                    W[name] = w

                load_r("pw0", (7, 512))
                load_r("qT", (7, MC), pool=io)
                load_r("pw1", (128, 4, 512))
                for h in HEADS:
                    load_r(h + "w0", (7, 256))
                    load_r(h + "w1", (128, 2, 256))
                load_r("pw1t", (128, 4, 512))
                load_r("pw0t", (128, 4, 7))
                load_r("pw2", (128, 4, 3))
                for h in HEADS:
                    load_r(h + "w1t", (128, 2, 256))
                    load_r(h + "w0t", (128, 2, 7))
                    load_r(h + "w2", (128, 2, 2))
                for n in ("sely", "selbc", "selbn", "sel7",
                          "blkA", "blkB", "blkC"):
                    load_r(n, dram[n].shape)
                load_f("pb0", (128, 4)), load_f("pb1", (128, 4))
                load_f("pb2", (3, 1))
                for h in HEADS:
                    load_f(h + "b0", (128, 2)), load_f(h + "b1", (128, 2))
                    load_f(h + "b2", (2, 1))
                load_f("qd4", (128, MC))

            # persistent J staging (quadrants 0/32/64 hold row blocks)
            jt = [io.tile([128, S], F32, tag=f"jt{i}", name=f"jt{i}")
                  for i in range(3)]
            jq = [io.tile([128, S], F32R, tag=f"jq{i}", name=f"jq{i}")
                  for i in range(3)]
            jaQ = io.tile([128, S], F32, tag="jaQ")
            for i in range(3):
                nc.vector.memset(jt[i][:, :], 0.0)

            NETS = {"p": dict(pre="p", C=4, O=3)}
            for h in HEADS:
                NETS[h] = dict(pre=h, C=2, O=2)

            with (
                tc.tile_pool(name="hd", bufs=1) as hd,
                tc.tile_pool(name="hd1", bufs=1) as hd1,
                tc.tile_pool(name="vz", bufs=2) as vz,
                tc.tile_pool(name="vc", bufs=2) as vc,
                tc.tile_pool(name="sm", bufs=1) as sm,
                tc.tile_pool(name="sm1", bufs=1) as sm1,
                tc.tile_pool(name="psA", bufs=4, space="PSUM") as psA,
                tc.tile_pool(name="psR", bufs=4, space="PSUM") as psR,
            ):
                def tile_body(t):
                    ts = slice(t * S, (t + 1) * S)
                    qr = W["qT"][:, ts]

                    def fwd(p):
                        net = NETS[p]
                        C, O = net["C"], net["O"]
                        pre = net["pre"]
                        w0, w1, w2 = W[pre + "w0"], W[pre + "w1"], W[pre + "w2"]
                        b0, b1 = W[pre + "b0"], W[pre + "b1"]
                        h1tag = "rh1" if pre != "p" else "ph1"
                        H1 = hd.tile([128, C, S], F32R, tag=h1tag)
                        D1 = hd1.tile([128, C, S], F32, tag=pre + "d1")
                        for jc in range(C):
                            z = psA.tile([128, S], F32, tag="psA")
                            nc.tensor.matmul(z[:, :],
                                             w0[0:7, jc * 128:(jc + 1) * 128],
                                             qr, start=True, stop=True)
                            nc.scalar.activation(H1[:, jc, :], z[:, :],
                                                 AF.Sigmoid,
                                                 bias=b0[:, jc:jc + 1])
                            nc.vector.scalar_tensor_tensor(
                                D1[:, jc, :], H1[:, jc, :], 1.0, H1[:, jc, :],
                                op0=ALU.subtract, op1=ALU.mult)
                        D2 = hd1.tile([128, C, S], F32, tag=pre + "d2")
                        ph = psR.tile([O, S], F32, tag="psR")
                        for kc in range(C):
                            z = psA.tile([128, S], F32, tag="psA")
                            for jc in range(C):
                                nc.tensor.matmul(
                                    z[:, :], w1[:, jc, kc * 128:(kc + 1) * 128],
                                    H1[:, jc, :], start=(jc == 0),
                                    stop=(jc == C - 1))
                            h2 = vz.tile([128, S], F32R, tag=pre + "h2")
                            nc.scalar.activation(h2[:, :], z[:, :], AF.Sigmoid,
                                                 bias=b1[:, kc:kc + 1])
                            nc.tensor.matmul(ph[:, :], w2[:, kc, 0:O],
                                             h2[:, :], start=(kc == 0),
                                             stop=(kc == C - 1))
                            nc.vector.scalar_tensor_tensor(
                                D2[:, kc, :], h2[:, :], 1.0, h2[:, :],
                                op0=ALU.subtract, op1=ALU.mult)
                        return D1, D2, ph

                    def bwd_stream(p, o, D1, D2, scale_ps):
                        net = NETS[p]
                        C, pre = net["C"], net["pre"]
                        w1t, w0t = W[pre + "w1t"], W[pre + "w0t"]
                        w2 = W[pre + "w2"]
                        vtag = "rvc" if pre != "p" else "pvc"
                        ztag = "rz" if pre != "p" else "pz"
                        Vs = []
                        for kc in range(C):
                            wcol = w2[:, kc, o:o + 1].bitcast(F32)
                            V = vc.tile([128, S], F32R, tag=vtag,
                                        bufs=(5 if pre == "p" else 3))
                            if scale_ps is None:
                                nc.scalar.activation(V[:, :], D2[:, kc, :],
                                                     AF.Identity, scale=wcol)
                            else:
                                nc.vector.scalar_tensor_tensor(
                                    V[:, :], D2[:, kc, :], wcol,
                                    scale_ps[:, :], op0=ALU.mult, op1=ALU.mult)
                            Vs.append(V)
                        jr = psR.tile([7, S], F32, tag="psR")
                        for jc in range(C):
                            y = psA.tile([128, S], F32, tag="psA")
                            for kc in range(C):
                                nc.tensor.matmul(
                                    y[:, :],
                                    w1t[:, kc, jc * 128:(jc + 1) * 128],
                                    Vs[kc][:, :], start=(kc == 0),
                                    stop=(kc == C - 1))
                            zc = vc.tile([128, S], F32R, tag=ztag)
                            nc.vector.tensor_tensor(zc[:, :], y[:, :],
                                                    D1[:, jc, :], op=ALU.mult)
                            nc.tensor.matmul(jr[:, :], w0t[:, jc, 0:7],
                                             zc[:, :], start=(jc == 0),
                                             stop=(jc == C - 1))
                        return jr

                    # ---------------- pos net ----------------
                    D1, D2, ph = fwd("p")
                    p3 = sm1.tile([3, S], F32, tag="p3")
                    nc.scalar.activation(p3[:, :], ph[:, :], AF.Identity,
                                         bias=W["pb2"][:, 0:1])
                    nc.sync.dma_start(dram["outT"][0:3, ts], p3[:, :])
                    nc.sync.dma_start(dram["outaT"][0:3, ts], p3[:, :])
                    for o in range(3):
                        jr = bwd_stream("p", o, D1, D2, None)
                        nc.scalar.copy(jt[0][32 * o:32 * o + 7, :], jr[:, :])

                    # ------------- rpy forward + batched trig -------------
                    rpy_fwd = {}
                    ypack = psR.tile([6, S], F32, tag="psR")
                    for hi, h in enumerate(HEADS):
                        D1h, D2h, phh = fwd(h)
                        rpy_fwd[h] = (D1h, D2h)
                        yh = sm1.tile([2, S], F32R, tag="yh")
                        nc.scalar.activation(yh[:, :], phh[:, :], AF.Identity,
                                             bias=W[h + "b2"][:, 0:1])
                        nc.tensor.matmul(ypack[:, :],
                                         W["sely"][:, 6 * hi:6 * hi + 6],
                                         yh[:, :], start=(hi == 0),
                                         stop=(hi == 2))
                    y6 = sm1.tile([6, S], F32, tag="y6")
                    nc.scalar.copy(y6[:, :], ypack[:, :])
                    yws = sm1.tile([6, S], F32, tag="yws")
                    nc.vector.add_range_wrap(yws[:, :], y6[:, :], 0.0, PI,
                                             2 * PI)
                    u6 = sm1.tile([6, S], F32, tag="u6")
                    nc.scalar.activation(u6[:, :], yws[:, :], AF.Square)
                    qq = [sm1.tile([6, S], F32, tag="qq0", name="qq0"),
                          sm1.tile([6, S], F32, tag="qq1", name="qq1")]
                    nc.vector.tensor_scalar_mul(qq[0][:, :], u6[:, :], QC[8])
                    b = 0
                    for k in range(7, 0, -1):
                        nc.vector.scalar_tensor_tensor(
                            qq[1 - b][:, :], qq[b][:, :], QC[k], u6[:, :],
                            op0=ALU.add, op1=ALU.mult)
                        b = 1 - b
                    # cos6: rows 0-2 cos(y0), rows 3-5 cos(y1)=c
                    cos6 = sm.tile([6, S], F32R, tag="cos6")
                    nc.vector.tensor_scalar(cos6[:, :], qq[b][:, :], QC[0],
                                            None, op0=ALU.add)
                    pp = [sm1.tile([6, S], F32, tag="qq0", name="pp0"),
                          sm1.tile([6, S], F32, tag="qq1", name="pp1")]
                    nc.vector.tensor_scalar_mul(pp[0][:, :], u6[:, :], PC[8])
                    a = 0
                    for k in range(7, 0, -1):
                        nc.vector.scalar_tensor_tensor(
                            pp[1 - a][:, :], pp[a][:, :], PC[k], u6[:, :],
                            op0=ALU.add, op1=ALU.mult)
                        a = 1 - a
                    # sin6: rows 0-2 sin(y0)=s, rows 3-5 sin(y1)
                    sin6 = sm.tile([6, S], F32R, tag="sin6")
                    nc.vector.scalar_tensor_tensor(sin6[:, :], pp[a][:, :],
                                                   PC[0], yws[:, :],
                                                   op0=ALU.add, op1=ALU.mult)
                    s3f = sin6[0:3, :].bitcast(F32)
                    nc.sync.dma_start(dram["outT"][3:6, ts],
                                      sin6[0:3, :].bitcast(F32))
                    nc.sync.dma_start(dram["outT"][6:9, ts],
                                      cos6[3:6, :].bitcast(F32))
                    c3f_t = sm1.tile([3, S], F32, tag="c3f")
                    nc.sync.dma_start(c3f_t[:, :],
                                      cos6[3:6, :].bitcast(F32))
                    c3f = c3f_t[:, :]

                    # angles: atan2(s, c) = atan(s/c) + pi*sign(s)*[c<0]
                    ssq = sm1.tile([3, S], F32, tag="t3a", name="ssq")
                    csq = sm1.tile([3, S], F32, tag="t3b", name="csq")
                    nc.scalar.activation(ssq[:, :], s3f, AF.Square)
                    nc.scalar.activation(csq[:, :], c3f, AF.Square)
                    rden = sm1.tile([3, S], F32, tag="t3c", name="rden")
                    nc.vector.tensor_tensor(rden[:, :], ssq[:, :], csq[:, :],
                                            op=ALU.add)
                    rinv = sm1.tile([3, S], F32, tag="rinv")
                    nc.vector.reciprocal(rinv[:, :], rden[:, :])
                    crec = sm1.tile([3, S], F32, tag="t3a", name="crec")
                    nc.vector.reciprocal(crec[:, :], c3f)
                    q3v = sm1.tile([3, S], F32, tag="t3b", name="q3v")
                    nc.vector.tensor_tensor(q3v[:, :], s3f, crec[:, :],
                                            op=ALU.mult)
                    at3 = sm1.tile([3, S], F32, tag="t3a", name="at3")
                    nc.scalar.activation(at3[:, :], q3v[:, :], AF.Arctan)
                    sgn3 = sm1.tile([3, S], F32, tag="t3c", name="sgn3")
                    nc.scalar.activation(sgn3[:, :], s3f, AF.Sign)
                    cneg = sm1.tile([3, S], F32, tag="t3d", name="cneg")
                    nc.vector.tensor_scalar(cneg[:, :], c3f, 0.0, None,
                                            op0=ALU.is_lt)
                    adj3 = sm1.tile([3, S], F32, tag="t3b", name="adj3")
                    nc.vector.scalar_tensor_tensor(adj3[:, :], sgn3[:, :], PI,
                                                   cneg[:, :], op0=ALU.mult,
                                                   op1=ALU.mult)
                    ang = sm1.tile([3, S], F32, tag="t3c", name="ang")
                    nc.vector.tensor_tensor(ang[:, :], at3[:, :], adj3[:, :],
                                            op=ALU.add)
                    nc.sync.dma_start(dram["outaT"][3:6, ts], ang[:, :])
                    cr3 = sm1.tile([3, S], F32R, tag="cr3")
                    sr3 = sm1.tile([3, S], F32R, tag="sr3")
                    nc.vector.tensor_tensor(cr3[:, :], c3f, rinv[:, :],
                                            op=ALU.mult)
                    nc.vector.tensor_tensor(sr3[:, :], s3f, rinv[:, :],
                                            op=ALU.mult)

                    # ---------------- rpy backward ----------------
                    tmpA = sm1.tile([128, S], F32, tag="tmpA")
                    tmpB = sm1.tile([128, S], F32, tag="tmpB")
                    for hi, h in enumerate(HEADS):
                        D1h, D2h = rpy_fwd[h]
                        qb = 32 * hi
                        c0b = psA.tile([128, S], F32, tag="psA")
                        nc.tensor.matmul(c0b[:, :],
                                         W["selbc"][:, 128 * hi:128 * hi + 128],
                                         cos6[:, :], start=True, stop=True)
                        s1b = psA.tile([128, S], F32, tag="psA")
                        nc.tensor.matmul(s1b[:, :],
                                         W["selbn"][:, 128 * hi:128 * hi + 128],
                                         sin6[:, :], start=True, stop=True)
                        jr_sin = bwd_stream(h, 0, D1h, D2h, c0b)
                        nc.scalar.copy(jt[1][qb:qb + 7, :], jr_sin[:, :])
                        crb = psR.tile([128, S], F32, tag="psR")
                        nc.tensor.matmul(crb[0:71, :],
                                         W["sel7"][:, 71 * hi:71 * hi + 71],
                                         cr3[:, :], start=True, stop=True)
                        nc.vector.tensor_tensor(tmpA[qb:qb + 7, :],
                                                jt[1][qb:qb + 7, :],
                                                crb[qb:qb + 7, :], op=ALU.mult)
                        jr_cos = bwd_stream(h, 1, D1h, D2h, s1b)
                        nc.scalar.copy(jt[2][qb:qb + 7, :], jr_cos[:, :])
                        srb = psR.tile([128, S], F32, tag="psR")
                        nc.tensor.matmul(srb[0:71, :],
                                         W["sel7"][:, 71 * hi:71 * hi + 71],
                                         sr3[:, :], start=True, stop=True)
                        nc.vector.tensor_tensor(tmpB[qb:qb + 7, :],
                                                jt[2][qb:qb + 7, :],
                                                srb[qb:qb + 7, :], op=ALU.mult)
                        nc.vector.tensor_tensor(jaQ[qb:qb + 7, :],
                                                tmpA[qb:qb + 7, :],
                                                tmpB[qb:qb + 7, :],
                                                op=ALU.subtract)

                    # ---------------- vel + output DMAs ----------------
                    for i in range(3):
                        nc.vector.tensor_tensor(jq[i][0:71, :],
                                                jt[i][0:71, :],
                                                W["qd4"][0:71, ts],
                                                op=ALU.mult)
                    vel = psR.tile([9, S], F32, tag="psR")
                    for i, bname in enumerate(("blkA", "blkB", "blkC")):
                        nc.tensor.matmul(vel[:, :], W[bname][:, 0:9],
                                         jq[i][0:71, :], start=(i == 0),
                                         stop=(i == 2))
                    velf = sm1.tile([9, S], F32, tag="p3", name="velf")
                    nc.scalar.copy(velf[:, :], vel[:, :])
                    nc.sync.dma_start(dram["outT"][9:18, ts], velf[:, :])
                    for q in range(3):
                        sl = jt[0][32 * q:32 * q + 7, :]
                        nc.sync.dma_start(
                            dram["jtT"][7 * q:7 * q + 7, ts], sl)
                        nc.sync.dma_start(
                            dram["jaT"][7 * q:7 * q + 7, ts], sl)
                        nc.sync.dma_start(
                            dram["jtT"][21 + 7 * q:28 + 7 * q, ts],
                            jt[1][32 * q:32 * q + 7, :])
                        nc.sync.dma_start(
                            dram["jtT"][42 + 7 * q:49 + 7 * q, ts],
                            jt[2][32 * q:32 * q + 7, :])
                        nc.sync.dma_start(
                            dram["jaT"][21 + 7 * q:28 + 7 * q, ts],
                            jaQ[32 * q:32 * q + 7, :])

                if reps > 4:
                    with tc.For_i(0, reps, 1):
                        for t in range(NT):
                            tile_body(t)
                else:
                    for _ in range(reps):
                        for t in range(NT):
                            tile_body(t)

    nc.compile()
    return nc


def _host_inputs(inputs):
    """Prepare per-core in_maps: shard x, replicate + re-lay-out weights."""
    def chunked(a, c):
        f = a.shape[1]
        return np.ascontiguousarray(
            a.reshape(c, 128, f).transpose(1, 0, 2)).astype(np.float32)

    com = {}
    for p, pre, d, o in (("pos", "p", 512, 3),) + tuple(
            (h, h, 256, 2) for h in HEADS):
        W0 = np.asarray(inputs[p + "_W0"], np.float32)
        W1 = np.asarray(inputs[p + "_W1"], np.float32)
        W2 = np.asarray(inputs[p + "_W2"], np.float32)
        b0 = np.asarray(inputs[p + "_b0"], np.float32)
        b1 = np.asarray(inputs[p + "_b1"], np.float32)
        b2 = np.asarray(inputs[p + "_b2"], np.float32)
        c = d // 128
        com[pre + "w0"] = np.ascontiguousarray(W0)
        com[pre + "w0t"] = chunked(np.ascontiguousarray(W0.T), c)
        com[pre + "w1"] = chunked(W1, c)
        com[pre + "w1t"] = chunked(np.ascontiguousarray(W1.T), c)
        com[pre + "w2"] = chunked(W2, c)
        com[pre + "b0"] = np.ascontiguousarray(b0.reshape(c, 128).T)
        com[pre + "b1"] = np.ascontiguousarray(b1.reshape(c, 128).T)
        com[pre + "b2"] = b2.reshape(o, 1).copy()

    sely = np.zeros((2, 18), np.float32)
    selbc = np.zeros((6, 384), np.float32)
    selbn = np.zeros((6, 384), np.float32)
    sel7 = np.zeros((3, 213), np.float32)
    for hi in range(3):
        sely[0, 6 * hi + hi] = 1.0
        sely[1, 6 * hi + 3 + hi] = 1.0
        selbc[hi, 128 * hi:128 * hi + 128] = 1.0
        selbn[3 + hi, 128 * hi:128 * hi + 128] = -1.0
        sel7[hi, 71 * hi + 32 * hi:71 * hi + 32 * hi + 7] = 1.0
    com["sely"], com["selbc"], com["selbn"], com["sel7"] = (sely, selbc,
                                                            selbn, sel7)
    for nm, gs in (("blkA", (0, 1, 2)), ("blkB", (3, 4, 5)),
                   ("blkC", (6, 7, 8))):
        blk = np.zeros((71, 9), np.float32)
        for q, g in enumerate(gs):
            blk[32 * q:32 * q + 7, g] = 1.0
        com[nm] = blk

    x = np.asarray(inputs["x"], np.float32)
    in_maps = []
    for cix in range(NCORES):
        xs = x[cix * MC:(cix + 1) * MC]            # (MC, 14)
        m = dict(com)
        m["qT"] = np.ascontiguousarray(xs[:, 0:7].T)
        qd4 = np.zeros((128, MC), np.float32)
        for q in range(4):
            qd4[32 * q:32 * q + 7] = xs[:, 7:14].T
        m["qd4"] = qd4
        in_maps.append(m)
    return in_maps


_PROGRAM = None


def _get_program():
    global _PROGRAM
    if _PROGRAM is None:
        _PROGRAM = _build(reps=1)
    return _PROGRAM


def run_spmd(nc, in_maps):
    from concourse.bass_utils import run_bass_kernel_spmd
    return run_bass_kernel_spmd(nc, in_maps, core_ids=list(range(NCORES)))


def _gather(results):
    out = np.concatenate([r["outT"].T for r in results], axis=0)
    j_tot = np.concatenate([r["jtT"].T for r in results], axis=0)
    j_tot = j_tot.reshape(M, 9, 7)
    out_ang = np.concatenate([r["outaT"].T for r in results], axis=0)
    j_ang = np.concatenate([r["jaT"].T for r in results], axis=0)
    j_ang = j_ang.reshape(M, 6, 7)
    return out, j_tot, out_ang, j_ang


def kernel(**inputs):
    nc = _get_program()
    in_maps = _host_inputs(inputs)
    res = run_spmd(nc, in_maps)
    return _gather(res.results)
